# revision 7
# baseline (speedup 1.0000x reference)
# DCN CrossLayer kernel for Trainium2 (8 NeuronCores, data-parallel over batch).
#
# Reference computation (per example row x of length D, L=3 layers):
#   cross = x
#   for i in range(L):
#       s_i   = <cross, W_i>                  (scalar per example)
#       cross = x * s_i + bias_i + cross
#
# Algebraic collapse: cross_i = a_i * x + B_i with per-example scalar a_i and
# batch-independent vector B_i = sum_{j<i} bias_j.  Then
#   s_i     = a_i * t_i + c_i,   t_i = <x, W_i>,  c_i = <B_i, W_i>
#   a_{i+1} = a_i * (1 + t_i) + c_i
#   out     = a_L * x + B_L
# so the device kernel only needs the three dot products t_i = <x, W_i>,
# a tiny per-row recurrence, and one per-row scale of x.  c_i and B_L are
# computed on the host (they do not depend on the batch).
#
# Measured HW facts this version is tuned around (perfetto):
#   - 16 DMA queues x ~25 GB/s each => ~400 GB/s/core aggregate; descriptor
#     cost scales with bytes, so fp16 I/O (4+4 MiB/core) floors DMA at ~21us
#   - PE matmul cost is ~flat (~215ns + 152ns ldweights) for N <= 512
#   - concurrently-enqueued DMAs round-robin-share the queues
#   - small DVE/ACT ops cost 0.4-0.7us each and cross-engine handoffs
#     ~0.5-1.5us, so the per-group chain (dots -> a3 -> broadcast -> scale
#     -> store) has ~5us latency.  Group sizes are shaped [256,512,512,
#     512,256]: a small head group gets the first output DMA going early
#     (outputs then interleave with remaining inputs on the queues), and a
#     small tail group shortens the end-of-kernel chain.
#   - the +1s ride the PSUM accumulation (ones-column matmul); recurrence
#     is one ACT pull of U0 plus two DVE muls reading U1/U2 straight from
#     PSUM at partitions 32/64 (quadrant rule; one PSUM operand per op)
#   - a3 is partition-broadcast per 256-row half on gpsimd (off PE/DVE),
#     ys/store are per-half and emitted with a one-group skew
import os
from contextlib import ExitStack

import numpy as np

import concourse.bacc as bacc
import concourse.bass as bass
import concourse.tile as tile
from concourse import mybir
from concourse.bass_utils import run_bass_kernel_spmd

B, D, L = 16384, 1024, 3
N_CORES = 8
ROWS = B // N_CORES  # rows per core
P = 128
KCH = D // P  # 8 d-chunks of 128
SIZES = [256, 512, 512, 512, 256]  # rows per dot-group (sum = ROWS)
H = 256  # scale/store half size
LPAD = 65  # zero-padded stationary width; layer l at column 32*l
GMAX = max(SIZES)

F32 = mybir.dt.float32
F16 = mybir.dt.float16

# test.py can flip these before calling kernel() to get an NTFF profile.
TRACE = False
LAST_RESULT = None


def _build(has_bias: bool, c1: float, c2: float) -> bass.Bass:
    nc = bacc.Bacc("TRN2", target_bir_lowering=False)
    xts = [
        nc.dram_tensor(f"xt{i}", [P, KCH, Gi], F16, kind="ExternalInput")
        for i, Gi in enumerate(SIZES)
    ]
    wt = nc.dram_tensor("wt", [P, KCH, LPAD], F16, kind="ExternalInput")
    w1 = nc.dram_tensor("w1", [1, LPAD], F16, kind="ExternalInput")
    if has_bias:
        bt = nc.dram_tensor("bt", [P, KCH], F32, kind="ExternalInput")
    yt = nc.dram_tensor("yt", [ROWS // H, P, KCH, H], F16, kind="ExternalOutput")

    with tile.TileContext(nc) as tc, ExitStack() as ctx:
        singles = ctx.enter_context(tc.tile_pool(name="singles", bufs=1))
        xpool = ctx.enter_context(tc.tile_pool(name="xpool", bufs=3))
        ypool = ctx.enter_context(tc.tile_pool(name="ypool", bufs=4))
        small = ctx.enter_context(tc.tile_pool(name="small", bufs=3))
        bpool = ctx.enter_context(tc.tile_pool(name="bpool", bufs=4))
        psT = ctx.enter_context(tc.tile_pool(name="psT", bufs=2, space="PSUM"))

        # tiny constant DMAs go on the SWDGE ring so they cannot delay the
        # first big x in-DMA on the SP HWDGE ring
        wt_sb = singles.tile([P, KCH, LPAD], F16)
        nc.gpsimd.dma_start(out=wt_sb, in_=wt[:])
        w1_sb = singles.tile([1, LPAD], F16)
        nc.gpsimd.dma_start(out=w1_sb, in_=w1[:])
        one_row = singles.tile([1, GMAX], F16)
        nc.vector.memset(one_row, 1.0)
        if has_bias:
            bt_sb = singles.tile([P, KCH], F32)
            nc.gpsimd.dma_start(out=bt_sb, in_=bt[:])

        NG = len(SIZES)
        KH = KCH // 2
        xs_t = [None] * NG
        pbh_t = [None] * NG
        row0_t = [0] * NG
        row0 = 0
        for g in range(NG + 1):
            if g >= 1:
                # skewed tail of group g-1: scale + store per 256-row half
                hg = g - 1
                Gh = SIZES[hg]
                for h in range(Gh // H):
                    ys = ypool.tile([P, KCH, H], F16, tag="ys")
                    pbh = pbh_t[hg][h]
                    pb_b = bass.AP(
                        tensor=pbh.tensor,
                        offset=pbh.offset,
                        ap=[pbh.ap[0], [0, KCH], pbh.ap[1]],
                    )
                    nc.vector.tensor_mul(
                        ys, xs_t[hg][:, :, h * H : (h + 1) * H], pb_b
                    )
                    if has_bias:
                        for k in range(KCH):
                            nc.vector.tensor_scalar_add(
                                ys[:, k, :], ys[:, k, :], bt_sb[:, k : k + 1]
                            )
                    # out-DMA on the ACT HWDGE ring
                    nc.scalar.dma_start(
                        out=yt[(row0_t[hg] + h * H) // H], in_=ys
                    )
            if g >= NG:
                break
            Gg = SIZES[g]
            row0_t[g] = row0
            xs = xpool.tile([P, KCH, Gg], F16, tag="xs")
            xs_t[g] = xs
            # split per 4-chunk half so the first matmuls can start after
            # only half the group has landed
            nc.sync.dma_start(out=xs[:, 0:KH, :], in_=xts[g][:, 0:KH, :])
            nc.sync.dma_start(out=xs[:, KH:KCH, :], in_=xts[g][:, KH:KCH, :])
            # U[32*l, j] = 1 + sum_d x[j, d] * W[l, d]; the +1 comes from a
            # ones-column matmul riding the same PSUM accumulation
            pt = psT.tile([LPAD, Gg], F32)
            for k in range(KCH):
                nc.tensor.matmul(
                    pt, wt_sb[:, k, :], xs[:, k, :], start=(k == 0), stop=False
                )
            nc.tensor.matmul(pt, w1_sb, one_row[:, :Gg], start=False, stop=True)
            # a3 = ((U0*U1)+c1)*U2 + c2  (c1 = c2 = 0 when bias is zero)
            ua = small.tile([1, Gg], F32, tag="ua")
            nc.scalar.copy(out=ua, in_=pt[0:1, :])
            a = small.tile([1, Gg], F32, tag="a")
            nc.vector.tensor_mul(a, ua, pt[32:33, :])
            if c1 != 0.0:
                nc.vector.tensor_scalar_add(a, a, c1)
            ah = small.tile([1, Gg], F16, tag="ah")
            if c2 != 0.0:
                a2 = small.tile([1, Gg], F32, tag="a2")
                nc.vector.tensor_mul(a2, a, pt[64:65, :])
                nc.vector.tensor_scalar_add(a2, a2, c2)
                nc.scalar.copy(out=ah, in_=a2)
            else:
                nc.vector.tensor_mul(ah, a, pt[64:65, :])
            # broadcast a3 to all partitions, per half (gpsimd, off the PE)
            pbh_t[g] = []
            for h in range(Gg // H):
                pbh = bpool.tile([P, H], F16, tag="pbh")
                pbh_t[g].append(pbh)
                nc.gpsimd.partition_broadcast(pbh, ah[0:1, h * H : (h + 1) * H])
            row0 += Gg
    nc.finalize()
    return nc


def kernel(x, W, bias):
    global LAST_RESULT
    x2 = np.asarray(x, dtype=np.float32).reshape(B, D)
    W2 = np.asarray(W, dtype=np.float32).reshape(L, D)
    B2 = np.asarray(bias, dtype=np.float32).reshape(L, D)

    # host-side constants
    has_bias = bool(np.any(B2 != 0.0))
    c1 = float(B2[0] @ W2[1])
    c2 = float((B2[0] + B2[1]) @ W2[2])
    b3 = B2.sum(axis=0)
    # wt[p, k, 32*l] = W[l, k*128 + p], zero elsewhere
    wt_host = np.zeros((P, KCH, LPAD), dtype=np.float16)
    wt_host[:, :, ::32] = W2.T.reshape(KCH, P, L).transpose(1, 0, 2)
    w1_host = np.zeros((1, LPAD), dtype=np.float16)
    w1_host[0, ::32] = 1.0
    # bt[p, k] = B_L[k*128 + p]
    bt_host = np.ascontiguousarray(b3.reshape(KCH, P).T)

    nc = _build(has_bias, c1 if has_bias else 0.0, c2 if has_bias else 0.0)

    in_maps = []
    for c in range(N_CORES):
        xc = x2[c * ROWS : (c + 1) * ROWS]
        # xt{g}[p, k, j] = xc[row0_g + j, k*128 + p]
        m = {"wt": wt_host, "w1": w1_host}
        r0 = 0
        for i, Gi in enumerate(SIZES):
            m[f"xt{i}"] = np.ascontiguousarray(
                xc[r0 : r0 + Gi]
                .reshape(Gi, KCH, P)
                .transpose(2, 1, 0)
                .astype(np.float16)
            )
            r0 += Gi
        if has_bias:
            m["bt"] = bt_host
        in_maps.append(m)

    kwargs = {}
    if TRACE:
        kwargs = dict(trace=True, trace_cores=[0])
    res = run_bass_kernel_spmd(nc, in_maps, core_ids=list(range(N_CORES)), **kwargs)
    LAST_RESULT = res
    out = np.empty((B, D), dtype=np.float32)
    for c in range(N_CORES):
        yt = res.results[c]["yt"]  # [ROWS//H, P, KCH, H] fp16
        # y[s*H + j, k*128 + p] = yt[s, p, k, j]
        out[c * ROWS : (c + 1) * ROWS] = (
            yt.transpose(0, 3, 2, 1).reshape(ROWS, D).astype(np.float32)
        )
    return np.ascontiguousarray(out.reshape(B, D, 1))
